# revision 3
# baseline (speedup 1.0000x reference)
"""GAT layer (PyG-style, add_self_loops=True) on 8 Trainium2 NeuronCores, v2.

Strategy: partition destination nodes (and their incident edges) across the 8
cores; each core owns a contiguous range of 6250 dst nodes.

Per core:
  phase 1: full projection table, 768 B bf16 rows in local DRAM (replicated
           compute).  Row layout (384 bf16 cols, last 120 are pad):
             [0:256]   h interleaved: col c = d*4+h  (d in 0..63, h in 0..3)
             [256:260] a_src (bf16)
             [260:264] a_dst (unused by phase 2; rides along in the one
                       384-col PSUM->SBUF copy)
             [264:384] zeros (WA is zero-padded so the copy covers full rows
                       and the 768 B table writes stay burst-aligned)
           Split into lo/hi tables (dma_gather idx is int16, in_ap base offset
           ignored by the Q7 ucode).
  phase 1b: a_dst for the core's own 6272 dst nodes (SBUF, bf16).
  phase 2: per window of 128 dst nodes, two dma_gathers (lo/hi tables,
           dummy-padded to compile-time-uniform subtile counts; gathers for a
           window PAIR share one tile and one index block) pull the src rows
           for all incident edges.
           One-hot (edge -> dst slot) built q-major vs a constant iota tile
           (2x DVE mode); PE transposes give ohT for the a_dst expansion;
           exp(leaky_relu(a_src+a_dst)) written straight into the msg tile;
           msg = h * e runs at DVE 2x (e broadcasts over the middle dim of
           the interleaved layout); segment-sum via bf16 PSUM-accumulated
           one-hot matmuls; out = acc/(denom+eps) + bias (un-interleaved by
           a strided read in the finalize mul).

Pad edges point at a dummy row with a_src = -1e30 => exp score exactly 0;
they also carry dstrel = -1 => one-hot column is all zero.

Host does index-space work (self-loop append, dst sort, windowing, padding,
int16 wrapping) plus small O(IN_DIM*HD) weight fusion and data layout.
"""

import math

import numpy as np

N = 50000
IN_DIM = 64
H = 4
D = 64
HD = H * D  # 256
ROWC = 384  # bf16 cols per table row = 768 B
WCOLS = 384  # phase-1 matmul output cols (zero-padded to a full row)
NEG_SLOPE = 0.2
EPS = 1e-16

NCORES = 8
NPC = N // NCORES  # 6250 dst nodes per core
NWIN = math.ceil(NPC / 128)  # 49 windows
WROWS = NWIN * 128  # 6272
NT1 = 392  # phase-1 tiles (50176 nodes incl. pad)
NROWS_ALL = NT1 * 128  # 50176
SPLIT_T = 196  # lo/hi table split, in 128-row tiles
SPLIT = SPLIT_T * 128  # 25088
LO_TILES = SPLIT_T + 1  # +1 dummy tile
LO_ROWS = LO_TILES * 128  # 25216
HI_TILES = NT1 - SPLIT_T  # 196
HI_ROWS = HI_TILES * 128  # 25088
DUMMY_LO = SPLIT  # row 25088 of lo table (dedicated dummy row)
DUMMY_HI = N - SPLIT  # row 24912 of hi table (= node 50000, h == 0)
SUBS = 4  # subtiles per transpose/copy chunk
B1 = 14  # phase-1 tiles per iteration (divides 392)
GBUFS = 4  # gather tile pool depth (window pairs in flight)
SINGLE_PACKET = False  # True wedges the device (Q7 ucode limit)
MB = 260  # msg block cols: [0:256] e*h interleaved, [256:260] e

LAST_RESULTS = None  # BassKernelResults of the most recent run (for test.py)

# column permutation: table col c (0..255) <- W col perm[c] (c = d*4+h)
_PERM = np.empty(256, np.int64)
for _c in range(256):
    _PERM[_c] = (_c % 4) * 64 + _c // 4


def _f32_to_bf16_bits(a):
    """Exact-for-small-ints f32 -> bf16 bit pattern (truncation)."""
    return (np.asarray(a, np.float32).view(np.uint32) >> 16).astype(np.uint16)


def _wrap_idx(ids):
    """[n] int -> dma_gather wrapped layout [128, n/16] int16
    (idx i at [i%16, i//16], replicated across the 8 Q7 core groups)."""
    n = len(ids)
    w16 = ids.reshape(n // 16, 16).T.astype(np.int16)  # [16, n/16]
    return np.tile(w16, (8, 1))


def _prep_host(edge_index):
    """Returns idx_host int16 [NCORES, 128, NWIN*C] (C = KL*8 + KH*8 + kj),
               KL, KH, cntL [NWIN], cntH [NWIN] (cross-core maxima)."""
    src = np.concatenate([edge_index[0], np.arange(N, dtype=np.int64)]).astype(np.int64)
    dst = np.concatenate([edge_index[1], np.arange(N, dtype=np.int64)]).astype(np.int64)
    order = np.argsort(dst, kind="stable")
    src = src[order].astype(np.int32)
    dst = dst[order].astype(np.int32)

    bounds = [c * NPC + w * 128 for c in range(NCORES) for w in range(NWIN)]
    bounds.append(N)
    cuts = np.searchsorted(dst, np.asarray(bounds))

    slo_all = {}
    shi_all = {}
    lo_counts = np.zeros((NCORES, NWIN), np.int64)
    hi_counts = np.zeros((NCORES, NWIN), np.int64)
    for c in range(NCORES):
        base = c * NPC
        for w in range(NWIN):
            b = c * NWIN + w
            s = src[cuts[b] : cuts[b + 1]]
            d = dst[cuts[b] : cuts[b + 1]] - base - w * 128
            m = s < SPLIT
            slo, dlo = s[m], d[m]
            shi, dhi = s[~m] - SPLIT, d[~m]
            o = np.argsort(slo, kind="stable")
            slo_all[c, w] = (slo[o], dlo[o])
            o = np.argsort(shi, kind="stable")
            shi_all[c, w] = (shi[o], dhi[o])
            lo_counts[c, w] = len(slo)
            hi_counts[c, w] = len(shi)

    KL = max(1, math.ceil(lo_counts.max() / 128))
    KH = max(1, math.ceil(hi_counts.max() / 128))
    kj = KL + KH
    # full dummy-padded gathers: the tile framework forbids reading tile
    # regions the current generation didn't write, so trimming trailing pad
    # indices (negative idx) is not safe; pads hit the dummy table row
    cntL = np.full(NWIN, KL * 128, np.int64)
    cntH = np.full(NWIN, KH * 128, np.int64)

    # gathers are merged per PAIR of windows (halves Q7 descriptor-gen fixed
    # cost); block layout per pair: [ilow(w)|ilow(w+1)] [ihigh(w)|ihigh(w+1)]
    # [drel(w)] [drel(w+1)]
    NP2 = (NWIN + 1) // 2
    C = 2 * (KL * 8 + KH * 8 + kj)
    idx_host = np.zeros((NCORES, 128, NP2 * C), np.int16)
    for c in range(NCORES):
        for wp in range(NP2):
            wins = [w for w in (2 * wp, 2 * wp + 1) if w < NWIN]
            nwp = len(wins)
            ilow = np.full((2, KL * 128), DUMMY_LO, np.int32)
            ihigh = np.full((2, KH * 128), DUMMY_HI, np.int32)
            drel = np.full((2, 128, kj), -1.0, np.float32)
            for u, w in enumerate(wins):
                slo, dlo = slo_all[c, w]
                shi, dhi = shi_all[c, w]
                ilow[u, : len(slo)] = slo
                ihigh[u, : len(shi)] = shi
                i = np.arange(len(slo))
                drel[u, i % 128, i // 128] = dlo
                i = np.arange(len(shi))
                drel[u, i % 128, KL + i // 128] = dhi
            blk = idx_host[c, :, wp * C : (wp + 1) * C]
            o = nwp * KL * 8
            blk[:, 0:o] = _wrap_idx(ilow[:nwp].reshape(-1))
            blk[:, 2 * KL * 8 : 2 * KL * 8 + nwp * KH * 8] = _wrap_idx(
                ihigh[:nwp].reshape(-1)
            )
            do = 2 * (KL + KH) * 8
            blk[:, do : do + kj] = _f32_to_bf16_bits(drel[0]).view(np.int16)
            blk[:, do + kj : do + 2 * kj] = _f32_to_bf16_bits(drel[1]).view(np.int16)
    return idx_host, KL, KH, cntL.tolist(), cntH.tolist()


def _make_wa(W, att_src, att_dst):
    """Host-fused phase-1 weight matrix [IN_DIM, 384] f32 (zero-padded):
    [0:256] column-permuted W, [256:260] Wsrc, [260:264] Wdst, [264:384] 0."""
    W = np.asarray(W, np.float64)
    a_s = np.asarray(att_src, np.float64)
    a_d = np.asarray(att_dst, np.float64)
    wa = np.zeros((IN_DIM, WCOLS), np.float64)
    wa[:, 0:256] = W[:, _PERM]
    for h in range(H):
        wa[:, 256 + h] = W[:, h * D : (h + 1) * D] @ a_s[h]
        wa[:, 260 + h] = W[:, h * D : (h + 1) * D] @ a_d[h]
    return wa.astype(np.float32)


def _build_program(KL, KH, cntL, cntH, ablate="full"):
    import concourse.bass as bass
    import concourse.bacc as bacc
    import concourse.tile as tile
    from concourse import mybir
    from concourse.masks import make_identity

    f32 = mybir.dt.float32
    bf16 = mybir.dt.bfloat16
    f8 = mybir.dt.float8e4
    i16 = mybir.dt.int16
    i32 = mybir.dt.int32
    kj = KL + KH
    NP2 = (NWIN + 1) // 2
    C = 2 * (KL * 8 + KH * 8 + kj)

    nc = bacc.Bacc(None, target_bir_lowering=False)

    xT_d = nc.dram_tensor("xT", [IN_DIM, NROWS_ALL], bf16, kind="ExternalInput")
    xdT_d = nc.dram_tensor("xdstT", [IN_DIM, WROWS], bf16, kind="ExternalInput")
    WA_d = nc.dram_tensor("WA", [IN_DIM, WCOLS], bf16, kind="ExternalInput")
    bias_d = nc.dram_tensor("bias", [1, HD], f32, kind="ExternalInput")
    idx_d = nc.dram_tensor("idx", [128, NP2 * C], i16, kind="ExternalInput")
    out_d = nc.dram_tensor("out", [WROWS, HD], f32, kind="ExternalOutput")
    tbl_lo = nc.dram_tensor("tbl_lo", [LO_ROWS, ROWC], bf16)  # 512 B rows
    tbl_hi = nc.dram_tensor("tbl_hi", [HI_ROWS, ROWC], bf16)

    EQ = mybir.AluOpType.is_equal
    MULT = mybir.AluOpType.mult
    MAX = mybir.AluOpType.max

    with tile.TileContext(nc) as tc:
        with tc.tile_pool(name="const", bufs=1) as cpool:
            spsum_cm = tc.tile_pool(name="setup_psum", bufs=1, space="PSUM")
            spsum = spsum_cm.__enter__()
            ones = cpool.tile([1, 128], f32)
            nc.vector.memset(ones[:], 1.0)
            ident_f = cpool.tile([128, 128], f32)
            make_identity(nc, ident_f[:])
            ident = cpool.tile([128, 128], bf16)
            nc.vector.tensor_copy(ident[:], ident_f[:])
            # iotaq[p, q*kj + s] = q  (constant per q-run of kj cols)
            iotaq_i = cpool.tile([128, 128 * kj], i32)
            nc.gpsimd.iota(iotaq_i[:], pattern=[[1, 128], [0, kj]], base=0,
                           channel_multiplier=0)
            iotaq = cpool.tile([128, 128 * kj], bf16)
            nc.vector.tensor_copy(iotaq[:], iotaq_i[:])

            # preload all phase-2 index/drel data in one DMA
            idx_sb = cpool.tile([128, NP2 * C], i16)
            nc.sync.dma_start(idx_sb[:], idx_d[:, :])

            WA_raw = cpool.tile([IN_DIM, WCOLS], bf16)
            nc.sync.dma_start(WA_raw[:], WA_d[:, :])
            WA = cpool.tile([IN_DIM, WCOLS], bf16)
            nc.vector.tensor_copy(WA[:], WA_raw[:])

            bias_raw = cpool.tile([1, HD], f32)
            nc.sync.dma_start(bias_raw[:], bias_d[:, :])
            bias_sb = cpool.tile([1, HD], f32)
            nc.vector.tensor_copy(bias_sb[:], bias_raw[:])
            bb = spsum.tile([128, HD], f32)
            nc.tensor.matmul(bb[:], lhsT=ones[:1, :], rhs=bias_sb[:], start=True, stop=True)
            bias_bc = cpool.tile([128, HD], f32)
            nc.scalar.copy(bias_bc[:], bb[:])

            # a_dst for the core's own dst shard: [128, NWIN*H] bf16 in SBUF
            adst_all = cpool.tile([128, NWIN * H], bf16)

            spsum_cm.__exit__(None, None, None)  # free setup PSUM banks

            # ---------------- phase 1: build src tables ----------------
            with (
                tc.tile_pool(name="p1", bufs=4) as p1,
                tc.tile_pool(name="p1ps", bufs=4, space="PSUM") as p1ps,
                tc.tile_pool(name="p1psb", bufs=2, space="PSUM") as p1psb,
            ):
                for it in range(NT1 // B1):
                    t0 = it * B1
                    xt = p1.tile([IN_DIM, B1 * 128], bf16, tag="xtr")
                    nc.sync.dma_start(xt[:], xT_d[:, t0 * 128 : (t0 + B1) * 128])
                    hs = p1.tile([128, B1 * ROWC], bf16, tag="hs")
                    for k in range(B1):
                        hp = p1ps.tile([128, WCOLS], f32, tag="hp")
                        nc.tensor.matmul(
                            hp[:],
                            lhsT=xt[:, k * 128 : (k + 1) * 128],
                            rhs=WA[:],
                            start=True,
                            stop=True,
                        )
                        if k % 2 == 0:
                            cp = nc.vector.tensor_copy
                        else:
                            cp = nc.scalar.copy
                        # one full-row copy: h | a_src | a_dst | zero pad
                        cp(hs[:, k * ROWC : (k + 1) * ROWC], hp[:])
                    if t0 < SPLIT_T:
                        dst_ap = tbl_lo[t0 * 128 : (t0 + B1) * 128, :]
                    else:
                        u = t0 - SPLIT_T
                        dst_ap = tbl_hi[u * 128 : (u + B1) * 128, :]
                    nc.sync.dma_start(
                        dst_ap.rearrange("(k p) c -> p k c", p=128),
                        hs[:].rearrange("p (k c) -> p k c", c=ROWC),
                    )

                # phase 1b: a_dst for own dst shard (stays in SBUF)
                xd = p1.tile([IN_DIM, WROWS], bf16, tag="xdr")
                nc.sync.dma_start(xd[:], xdT_d[:, :])
                adp = p1psb.tile([128, NWIN * H], f32, tag="adp")
                for w in range(NWIN):
                    nc.tensor.matmul(
                        adp[:, w * H : (w + 1) * H],
                        lhsT=xd[:, w * 128 : (w + 1) * 128],
                        rhs=WA[:, HD + H : HD + 2 * H],
                        start=True, stop=True,
                    )
                nc.vector.tensor_copy(adst_all[:], adp[:])

            # dummy rows: whole trailing lo tile; a_src of hi dummy (node 50000)
            zrow = cpool.tile([128, ROWC], bf16)
            nc.vector.memset(zrow[:], 0.0)
            nc.vector.memset(zrow[:, 256:260], -1e30)
            nc.sync.dma_start(tbl_lo[DUMMY_LO : DUMMY_LO + 128, :], zrow[:])
            nc.sync.dma_start(
                tbl_hi[DUMMY_HI : DUMMY_HI + 1, 256:260], zrow[:1, 256:260]
            )

            # ---------------- phase 2: per-window aggregation ----------------
            if ablate == "p1":
                with tc.tile_pool(name="fin0", bufs=1) as f0:
                    zo = f0.tile([128, HD], f32)
                    nc.vector.memset(zo[:], 0.0)
                    for w in range(NWIN):
                        nc.sync.dma_start(out_d[w * 128 : (w + 1) * 128, :], zo[:])
            nch = math.ceil(kj / SUBS)
            with (
                tc.tile_pool(name="gat", bufs=GBUFS) as gpool,
                tc.tile_pool(name="wrk", bufs=2) as wpool,
                tc.tile_pool(name="fin", bufs=2) as fpool,
                tc.tile_pool(name="accps", bufs=2, space="PSUM") as accps,
                tc.tile_pool(name="trps", bufs=2, space="PSUM") as trps,
                tc.tile_pool(name="adxps", bufs=2, space="PSUM") as adxps,
            ):
                NP2r = (NWIN + 1) // 2 if ablate != "p1" else 0
                for wp in range(NP2r):
                  nwp = 2 if 2 * wp + 1 < NWIN else 1
                  blk = wp * C
                  # pair tile layout: [w0-lo | w1-lo | w0-hi | w1-hi]
                  # (gather out must be contiguous, so a window's lo and hi
                  # subtiles are two separate views downstream)
                  g = gpool.tile([128, 2 * kj * ROWC], bf16, tag="g")
                  gxv = g[:].rearrange("p (x c) -> p x c", c=ROWC)
                  # one gather per (window, table): large merged calls stall
                  # the SWDGE FIFO on hw
                  for u in range(nwp):
                      nc.gpsimd.dma_gather(
                          out_ap=gxv[:, u * KL : (u + 1) * KL, :],
                          in_ap=tbl_lo[:, :],
                          idxs_ap=idx_sb[
                              :, blk + u * KL * 8 : blk + (u + 1) * KL * 8
                          ],
                          num_idxs=KL * 128, num_idxs_reg=KL * 128,
                          elem_size=ROWC, single_packet=SINGLE_PACKET,
                      )
                      nc.gpsimd.dma_gather(
                          out_ap=gxv[:, 2 * KL + u * KH : 2 * KL + (u + 1) * KH, :],
                          in_ap=tbl_hi[:, :],
                          idxs_ap=idx_sb[
                              :,
                              blk + 2 * KL * 8 + u * KH * 8 : blk
                              + 2 * KL * 8
                              + (u + 1) * KH * 8,
                          ],
                          num_idxs=KH * 128, num_idxs_reg=KH * 128,
                          elem_size=ROWC, single_packet=SINGLE_PACKET,
                      )
                  for u in range(nwp):
                    w = 2 * wp + u
                    gvlo = gxv[:, u * KL : (u + 1) * KL, :]
                    gvhi = gxv[:, 2 * KL + u * KH : 2 * KL + (u + 1) * KH, :]
                    do = blk + 2 * (KL + KH) * 8
                    drel = idx_sb[:, do + u * kj : do + (u + 1) * kj].bitcast(bf16)
                    adw = adst_all[:, w * H : (w + 1) * H]

                    if ablate == "p1g":
                        outw = fpool.tile([128, HD], f32, tag="outw")
                        nc.vector.tensor_copy(outw[:], gvlo[:, 0, 0:HD])
                        nc.sync.dma_start(out_d[w * 128 : (w + 1) * 128, :], outw[:])
                        continue

                    # one-hot, q-major: oh[p, q, s] = (drel[p, s] == q), 2x DVE
                    oh = wpool.tile([128, 128 * kj], bf16, tag="oh")
                    nc.vector.tensor_tensor(
                        out=oh[:].rearrange("p (q s) -> p q s", s=kj),
                        in0=drel.unsqueeze(1).to_broadcast([128, 128, kj]),
                        in1=iotaq[:].rearrange("p (q s) -> p q s", s=kj),
                        op=EQ,
                    )
                    ohv = oh[:].rearrange("p (q s) -> p q s", s=kj)

                    # transposed one-hot via PE; PSUM -> SBUF bf16 on Act
                    ohT = wpool.tile([128, kj * 128], bf16, tag="ohT")
                    for ch in range(nch):
                        s0 = ch * SUBS
                        ns = min(SUBS, kj - s0)
                        ohT_ps = trps.tile([128, SUBS * 128], bf16, tag="ohTp")
                        for s in range(ns):
                            nc.tensor.transpose(
                                ohT_ps[:, s * 128 : (s + 1) * 128],
                                ohv[:, :, s0 + s],
                                ident[:],
                            )
                        nc.scalar.copy(
                            ohT[:, s0 * 128 : (s0 + ns) * 128],
                            ohT_ps[:, 0 : ns * 128],
                        )
                    # a_dst per edge: adx[p, s*H+h]
                    adx = adxps.tile([128, kj * H], f32, tag="adx")
                    for s in range(kj):
                        nc.tensor.matmul(
                            adx[:, s * H : (s + 1) * H],
                            lhsT=ohT[:, s * 128 : (s + 1) * 128],
                            rhs=adw,
                            start=True,
                            stop=True,
                        )
                    # scores: sc = a_src + a_dst ; lr = leaky_relu(sc)
                    sc = fpool.tile([128, kj * H], f32, tag="sc")
                    nc.vector.tensor_add(
                        sc[:, 0 : KL * H].rearrange("p (s h) -> p s h", h=H),
                        gvlo[:, :, 256:260],
                        adx[:, 0 : KL * H].rearrange("p (s h) -> p s h", h=H),
                    )
                    nc.vector.tensor_add(
                        sc[:, KL * H :].rearrange("p (s h) -> p s h", h=H),
                        gvhi[:, :, 256:260],
                        adx[:, KL * H :].rearrange("p (s h) -> p s h", h=H),
                    )
                    lr = fpool.tile([128, kj * H], f32, tag="lr")
                    nc.vector.scalar_tensor_tensor(
                        out=lr[:], in0=sc[:], scalar=NEG_SLOPE, in1=sc[:],
                        op0=MULT, op1=MAX,
                    )
                    # msg tile: [0:256] e*h (interleaved), [256:260] e
                    mv = wpool.tile([128, kj * MB], bf16, tag="mv")
                    mvv = mv[:].rearrange("p (s c) -> p s c", c=MB)
                    nc.scalar.activation(
                        mvv[:, :, 256:260],
                        lr[:].rearrange("p (s h) -> p s h", h=H),
                        mybir.ActivationFunctionType.Exp,
                    )
                    # msg: 2x DVE (e broadcasts over middle dim d)
                    nc.vector.tensor_tensor(
                        out=mvv[:, 0:KL, 0:256].rearrange(
                            "p s (d h) -> p s d h", h=H
                        ),
                        in0=gvlo[:, :, 0:256].rearrange("p s (d h) -> p s d h", h=H),
                        in1=mvv[:, 0:KL, 256:260].unsqueeze(2).to_broadcast(
                            [128, KL, 64, H]
                        ),
                        op=MULT,
                    )
                    nc.vector.tensor_tensor(
                        out=mvv[:, KL:kj, 0:256].rearrange(
                            "p s (d h) -> p s d h", h=H
                        ),
                        in0=gvhi[:, :, 0:256].rearrange("p s (d h) -> p s d h", h=H),
                        in1=mvv[:, KL:kj, 256:260].unsqueeze(2).to_broadcast(
                            [128, KH, 64, H]
                        ),
                        op=MULT,
                    )
                    # segment-sum: acc[slot, 0:256]=sum e*h ; [256:260]=sum e
                    acc = accps.tile([128, MB], f32, tag="acc")
                    for s in range(kj):
                        nc.tensor.matmul(
                            acc[:],
                            lhsT=ohv[:, :, s],
                            rhs=mvv[:, s, :],
                            start=(s == 0),
                            stop=(s == kj - 1),
                        )
                    # finalize: out = acc/(dns + eps) + bias, un-interleave
                    dnse = fpool.tile([128, H], f32, tag="dnse")
                    nc.vector.tensor_scalar_add(dnse[:], acc[:, 256:260], EPS)
                    dnr = fpool.tile([128, H], f32, tag="dnr")
                    nc.vector.reciprocal(dnr[:], dnse[:])
                    outw = fpool.tile([128, HD], f32, tag="outw")
                    nc.vector.tensor_tensor(
                        out=outw[:].rearrange("p (h d) -> p h d", d=D),
                        in0=acc[:, 0:256].rearrange("p (d h) -> p h d", h=H),
                        in1=dnr[:].unsqueeze(-1).to_broadcast([128, H, D]),
                        op=MULT,
                    )
                    nc.vector.tensor_add(outw[:], outw[:], bias_bc[:])
                    nc.sync.dma_start(out_d[w * 128 : (w + 1) * 128, :], outw[:])
    nc.compile()
    # compile()'s late passes can leave >1-wait instructions behind; one more
    # split pass clears them (TRN2 allows one sem wait per compute inst).
    nc.generate_event_semaphores()
    return nc


def _to_bf16(a):
    import ml_dtypes

    return np.asarray(a, np.float32).astype(ml_dtypes.bfloat16)


def build_for_timing(inputs, ablate="full"):
    """(nc, in_maps) for test.py's burst-timing path."""
    x = np.asarray(inputs["x"], dtype=np.float32)
    idx_host, KL, KH, cntL, cntH = _prep_host(np.asarray(inputs["edge_index"]))
    wa = _make_wa(inputs["W"], inputs["att_src"], inputs["att_dst"])
    import ml_dtypes

    xT = np.zeros((IN_DIM, NROWS_ALL), dtype=ml_dtypes.bfloat16)
    xT[:, :N] = _to_bf16(x.T)
    wa16 = _to_bf16(wa)
    bias_row = np.ascontiguousarray(
        np.asarray(inputs["bias"], np.float32).reshape(1, HD)
    )
    nc = _build_program(KL, KH, cntL, cntH, ablate=ablate)
    in_maps = []
    for c in range(NCORES):
        xdT = np.zeros((IN_DIM, WROWS), dtype=ml_dtypes.bfloat16)
        xdT[:, :NPC] = _to_bf16(x[c * NPC : (c + 1) * NPC].T)
        in_maps.append(
            {
                "xT": xT,
                "xdstT": np.ascontiguousarray(xdT),
                "WA": wa16,
                "bias": bias_row,
                "idx": np.ascontiguousarray(idx_host[c]),
            }
        )
    return nc, in_maps


def kernel(x, edge_index, W, att_src, att_dst, bias):
    global LAST_RESULTS
    from concourse.bass_utils import run_bass_kernel_spmd

    x = np.asarray(x, dtype=np.float32)
    edge_index = np.asarray(edge_index)
    W = np.asarray(W, dtype=np.float32)
    att_src = np.asarray(att_src, dtype=np.float32)
    att_dst = np.asarray(att_dst, dtype=np.float32)
    bias = np.asarray(bias, dtype=np.float32)

    idx_host, KL, KH, cntL, cntH = _prep_host(edge_index)
    wa = _make_wa(W, att_src, att_dst)

    import ml_dtypes

    xT = np.zeros((IN_DIM, NROWS_ALL), dtype=ml_dtypes.bfloat16)
    xT[:, :N] = _to_bf16(x.T)
    wa16 = _to_bf16(wa)
    bias_row = np.ascontiguousarray(bias.reshape(1, HD))

    nc = _build_program(KL, KH, cntL, cntH)

    in_maps = []
    for c in range(NCORES):
        xdT = np.zeros((IN_DIM, WROWS), dtype=ml_dtypes.bfloat16)
        xdT[:, :NPC] = _to_bf16(x[c * NPC : (c + 1) * NPC].T)
        in_maps.append(
            {
                "xT": xT,
                "xdstT": np.ascontiguousarray(xdT),
                "WA": wa16,
                "bias": bias_row,
                "idx": np.ascontiguousarray(idx_host[c]),
            }
        )

    res = run_bass_kernel_spmd(nc, in_maps, list(range(NCORES)))
    LAST_RESULTS = res

    out = np.empty((N, HD), dtype=np.float32)
    for c in range(NCORES):
        out[c * NPC : (c + 1) * NPC] = res.results[c]["out"][:NPC]
    return out
